# revision 1
# baseline (speedup 1.0000x reference)
"""Trainium2 Bass kernel for nn_Attention (dense transformer MHA block).

Reference computation (per batch element b of 8):
    qkv = x @ w_qkv;  q,k,v split into 16 heads of dim 64
    out = softmax(q k^T / 8) v  (per head),  y = out @ w_proj + b_proj

Sharding: pure data-parallel over the batch (B=8 == n_cores). Each core
computes one batch element's full attention with replicated weights; no
collectives. Full inputs in, full outputs out; gather = np.stack.

Per-core dataflow (MM_DTYPE matmuls, fp32 PSUM accumulate):
  1. per n-tile: load x, PE-transpose to xT [d,n]; v-pass matmuls
     (xT-stationary, w_v-moving) packed into v_aug [n, 16*(64+1)] with a
     ones column per head (integrated softmax denominator)
  2. per head pair: qT,kT [c,n] tiles via w-stationary / xT-moving matmuls
  3. per head: S_T[j,i] = kT-stationary @ qT-moving (K=64);
     P_T = exp(S_T/8) on ScalarE (PSUM -> SBUF, MM_DTYPE out);
     PV: outT[65,i] accumulated over j-chunks, v_aug-stationary,
     P_T-moving; row 64 = softmax denominator
  4. normalize: reciprocal (DVE) -> partition_broadcast (GpSimd, base-0
     tiles only) -> tensor_tensor multiply (DVE) into attn_outT [c,n]
  5. proj: attn_outT-stationary @ w_proj-moving; bias added during the
     PSUM->SBUF eviction via a pre-broadcast bias tile (DVE add)
"""

import numpy as np
from contextlib import ExitStack

import concourse.bass as bass
import concourse.bacc as bacc
import concourse.mybir as mybir
from concourse import tile
from concourse.bass_utils import run_bass_kernel_spmd
from concourse.masks import make_identity

F32 = mybir.dt.float32
F32R = mybir.dt.float32r
BF16 = mybir.dt.bfloat16
EXPF = mybir.ActivationFunctionType.Exp

MM_DTYPE = F32R   # matmul operand dtype: F32R (safe) or BF16 (fast?)

# tuning knobs (defaults = best known)
CONFIG = {
    "mm_bufs": 4,     # shared matmul-out PSUM pool bufs (1 bank each)
    "s_tile": 1024,   # S psum tile free size: 1024 (2 banks) or 512 (1 bank)
    "s_bufs": 2,
    "pv_bf16": False, # v_aug + P_T in bf16 (PV matmul in bf16)
    "p_bufs": 8,
    "xr_chunk": False,
}

N = 1024          # sequence length (per core)
D = 1024          # model dim
H = 16            # heads
HD = 64           # head dim
SCALE = HD ** -0.5
P = 128           # partitions
NT = N // P       # 8 n-tiles
DT = D // P       # 8 d-chunks
NCORES = 8


def _build(tc, nc, x_d, wqkv_d, wproj_d, bproj_d, y_d, phases="full"):
    mul = mybir.AluOpType.mult
    add = mybir.AluOpType.add
    MD = MM_DTYPE

    with ExitStack() as outer:
        const = outer.enter_context(tc.tile_pool(name="const", bufs=1))
        ones16 = const.tile([P, H], F32)
        nc.gpsimd.memset(ones16[:], 1.0)
        bias_bc = const.tile([P, D], F32)

        attn_pool = outer.enter_context(tc.tile_pool(name="attnout", bufs=DT))
        attn_t = [attn_pool.tile([P, N], MD, tag="attn", name=f"attn{i}")
                  for i in range(DT)]
        outsb = outer.enter_context(tc.tile_pool(name="outsb", bufs=2))
        mm_ps = outer.enter_context(
            tc.tile_pool(name="mmps", bufs=CONFIG["mm_bufs"], space="PSUM"))
        VD = BF16 if CONFIG["pv_bf16"] else MD

        xt_pool = outer.enter_context(tc.tile_pool(name="xT", bufs=DT))
        xT = [xt_pool.tile([P, N], MD, tag="xT", name=f"xT{i}")
              for i in range(DT)]
        vaug_pool = outer.enter_context(tc.tile_pool(name="vaug", bufs=NT))
        vaug = [vaug_pool.tile([P, H * (HD + 1)], VD, tag="vaug",
                               name=f"vaug{i}") for i in range(NT)]

        # ---- phase A: bias broadcast; per n-tile x->xT transpose + v-pass ----
        with ExitStack() as phA:
            scratch = phA.enter_context(tc.tile_pool(name="scratch", bufs=1))
            xload = phA.enter_context(tc.tile_pool(name="xload", bufs=2))
            xr_pool = phA.enter_context(tc.tile_pool(name="xr", bufs=2))
            wv_f = phA.enter_context(tc.tile_pool(name="wvf", bufs=2))
            wv_pool = phA.enter_context(tc.tile_pool(name="wvr", bufs=2 * DT))

            ident = scratch.tile([P, P], MD)
            ident_f = scratch.tile([P, P], F32)
            make_identity(nc, ident_f[:])
            nc.vector.tensor_copy(ident[:], ident_f[:])
            bstage = scratch.tile([1, D], F32)
            nc.sync.dma_start(bstage[:], bproj_d[:].rearrange("(a f) -> a f", a=1))
            nc.gpsimd.partition_broadcast(bias_bc[:], bstage[:])

            wv_r = {}
            for cv in range(2):
                for dt in range(DT):
                    wf = wv_f.tile([P, 512], F32)
                    nc.sync.dma_start(
                        wf[:], wqkv_d[dt * P:(dt + 1) * P,
                                      2 * D + cv * 512: 2 * D + (cv + 1) * 512])
                    wr = wv_pool.tile([P, 512], MD, tag="wv", name=f"wv{cv}_{dt}")
                    nc.vector.tensor_copy(wr[:], wf[:])
                    wv_r[(cv, dt)] = wr

            for nt in range(NT):
                xf = xload.tile([P, D], F32)
                nc.sync.dma_start(xf[:], x_d[nt * P:(nt + 1) * P, :])
                xr = xr_pool.tile([P, D], MD)
                if CONFIG["xr_chunk"]:
                    for dt in range(DT):
                        nc.vector.tensor_copy(xr[:, dt * P:(dt + 1) * P],
                                              xf[:, dt * P:(dt + 1) * P])
                else:
                    nc.vector.tensor_copy(xr[:], xf[:])
                for dt in range(DT):
                    tp = mm_ps.tile([P, P], MD, tag="mm")
                    nc.tensor.transpose(tp[:], xr[:, dt * P:(dt + 1) * P],
                                        ident[:])
                    nc.vector.tensor_copy(xT[dt][:, nt * P:(nt + 1) * P], tp[:])
                # ones columns, then v matmuls for this n-tile
                nc.vector.tensor_copy(
                    vaug[nt][:].rearrange("p (h e) -> p h e", h=H)[:, :, HD:HD + 1],
                    ones16[:].rearrange("p (h e) -> p h e", e=1))
                for cv in range(2):
                    vp = mm_ps.tile([P, 512], F32, tag="mm")
                    for dt in range(DT):
                        nc.tensor.matmul(
                            vp[:], xT[dt][:, nt * P:(nt + 1) * P],
                            wv_r[(cv, dt)][:], start=(dt == 0), stop=(dt == DT - 1))
                    dst = vaug[nt][:].rearrange(
                        "p (h e) -> p h e", h=H)[:, 8 * cv:8 * cv + 8, 0:HD]
                    src = vp[:].rearrange("p (h e) -> p h e", h=8)
                    nc.vector.tensor_copy(dst, src)

        if phases == "A":
            for nt in range(NT):
                yo = outsb.tile([P, 512], F32, tag="y")
                nc.vector.tensor_copy(yo[:], vaug[nt][:, 0:512])
                nc.sync.dma_start(y_d[nt * P:(nt + 1) * P, 0:512], yo[:])
            return

        # ---- phase B: per head pair qk + attention ----
        with ExitStack() as phB:
            wqk_f = phB.enter_context(tc.tile_pool(name="wqkf", bufs=3))
            wqk_pool = phB.enter_context(tc.tile_pool(name="wqkr", bufs=9))
            qk_pool = phB.enter_context(tc.tile_pool(name="qk", bufs=3))
            p_pool = phB.enter_context(
                tc.tile_pool(name="pT", bufs=CONFIG["p_bufs"]))
            s_ps = phB.enter_context(
                tc.tile_pool(name="sps", bufs=CONFIG["s_bufs"], space="PSUM"))
            rt_pool = phB.enter_context(tc.tile_pool(name="rt", bufs=2))
            bt_pool = phB.enter_context(tc.tile_pool(name="bt", bufs=2))

            for pair in range(H // 2):
                # w_qkv columns for q-tile `pair` and k-tile `pair`:
                # cols pair*128 + {0, 1024}; uniform stride 1024, count 2.
                wqk_r = []
                for dt in range(DT):
                    wf = wqk_f.tile([P, 2, P], F32)
                    src = wqkv_d[dt * P:(dt + 1) * P, :].rearrange(
                        "p (k r e) -> p k r e", k=3, e=P)[:, 0:2, pair, :]
                    nc.sync.dma_start(wf[:], src)
                    wr = wqk_pool.tile([P, 2, P], MD, tag="wqk",
                                       name=f"wqk{pair}_{dt}")
                    nc.vector.tensor_copy(wr[:], wf[:])
                    wqk_r.append(wr)
                qk_t = []
                for which in range(2):   # 0 = q, 1 = k
                    ct = qk_pool.tile([P, N], MD, tag="qk",
                                      name=f"qk{pair}_{which}")
                    for nch in range(2):
                        qp = mm_ps.tile([P, 512], F32, tag="mm")
                        for dt in range(DT):
                            nc.tensor.matmul(
                                qp[:], wqk_r[dt][:, which, :],
                                xT[dt][:, nch * 512:(nch + 1) * 512],
                                start=(dt == 0), stop=(dt == DT - 1))
                        nc.vector.tensor_copy(
                            ct[:, nch * 512:(nch + 1) * 512], qp[:])
                    qk_t.append(ct)
                qT, kT = qk_t

                for hh in range(2):
                    h = 2 * pair + hh
                    base = HD * hh
                    p_t = {}
                    if CONFIG["s_tile"] == 1024:
                        for jt in range(NT):
                            sp = s_ps.tile([P, N], F32, tag="s")
                            for ich in range(2):
                                nc.tensor.matmul(
                                    sp[:, ich * 512:(ich + 1) * 512],
                                    kT[base:base + HD, jt * P:(jt + 1) * P],
                                    qT[base:base + HD, ich * 512:(ich + 1) * 512],
                                    start=True, stop=True)
                            pt = p_pool.tile([P, N], VD, tag="p",
                                             name=f"pT{h}_{jt}")
                            nc.scalar.activation(pt[:], sp[:], EXPF, scale=SCALE)
                            for ich in range(2):
                                p_t[(jt, ich)] = pt[:, ich * 512:(ich + 1) * 512]
                    else:
                        for jt in range(NT):
                            for ich in range(2):
                                sp = s_ps.tile([P, 512], F32, tag="s")
                                nc.tensor.matmul(
                                    sp[:],
                                    kT[base:base + HD, jt * P:(jt + 1) * P],
                                    qT[base:base + HD, ich * 512:(ich + 1) * 512],
                                    start=True, stop=True)
                                pt = p_pool.tile([P, 512], VD, tag="p",
                                                 name=f"pT{h}_{jt}_{ich}")
                                nc.scalar.activation(pt[:], sp[:], EXPF,
                                                     scale=SCALE)
                                p_t[(jt, ich)] = pt[:]
                    rt = rt_pool.tile([1, N], F32, tag="rt")
                    bt = bt_pool.tile([HD, N], F32, tag="bt")
                    for ich in range(2):
                        pv = mm_ps.tile([HD + 1, 512], F32, tag="mm")
                        for jt in range(NT):
                            nc.tensor.matmul(
                                pv[:],
                                vaug[jt][:, h * (HD + 1):(h + 1) * (HD + 1)],
                                p_t[(jt, ich)],
                                start=(jt == 0), stop=(jt == NT - 1))
                        sl = slice(ich * 512, (ich + 1) * 512)
                        nc.vector.reciprocal(rt[:, sl], pv[HD:HD + 1, :])
                        nc.gpsimd.partition_broadcast(bt[:, sl], rt[:, sl])
                        nc.vector.tensor_tensor(
                            attn_t[h // 2][base:base + HD, sl],
                            pv[0:HD, :], bt[:, sl], mul)

        if phases == "AB":
            for cc in range(DT):
                yo = outsb.tile([P, 512], F32, tag="y")
                nc.vector.tensor_copy(yo[:], attn_t[cc][:, 0:512])
                nc.sync.dma_start(y_d[cc * P:(cc + 1) * P, 0:512], yo[:])
            return

        # ---- projection (+ bias via DVE add during eviction) ----
        with ExitStack() as ph3:
            wp_f = ph3.enter_context(tc.tile_pool(name="wpf", bufs=3))
            wp_pool = ph3.enter_context(tc.tile_pool(name="wpr", bufs=2 * DT))
            wp_r = {}
            for cc in range(DT):
                for ec in range(2):
                    wf = wp_f.tile([P, 512], F32)
                    nc.sync.dma_start(
                        wf[:], wproj_d[cc * P:(cc + 1) * P,
                                       ec * 512:(ec + 1) * 512])
                    wr = wp_pool.tile([P, 512], MD, tag="wp", name=f"wp{cc}_{ec}")
                    nc.vector.tensor_copy(wr[:], wf[:])
                    wp_r[(cc, ec)] = wr
            for nt in range(NT):
                for ec in range(2):
                    yp = mm_ps.tile([P, 512], F32, tag="mm")
                    for cc in range(DT):
                        nc.tensor.matmul(
                            yp[:], attn_t[cc][:, nt * P:(nt + 1) * P],
                            wp_r[(cc, ec)][:], start=(cc == 0),
                            stop=(cc == DT - 1))
                    yo = outsb.tile([P, 512], F32, tag="y")
                    nc.vector.tensor_tensor(
                        yo[:], yp[:], bias_bc[:, ec * 512:(ec + 1) * 512], add)
                    nc.sync.dma_start(
                        y_d[nt * P:(nt + 1) * P, ec * 512:(ec + 1) * 512], yo[:])


def build_nc(reps=1, phases="full"):
    nc = bacc.Bacc("TRN2", target_bir_lowering=False, debug=False)
    x_d = nc.dram_tensor("x", [N, D], F32, kind="ExternalInput").ap()
    wqkv_d = nc.dram_tensor("w_qkv", [D, 3 * D], F32, kind="ExternalInput").ap()
    wproj_d = nc.dram_tensor("w_proj", [D, D], F32, kind="ExternalInput").ap()
    bproj_d = nc.dram_tensor("b_proj", [D], F32, kind="ExternalInput").ap()
    y_d = nc.dram_tensor("y", [N, D], F32, kind="ExternalOutput").ap()
    with tile.TileContext(nc) as tc:
        for _ in range(reps):
            _build(tc, nc, x_d, wqkv_d, wproj_d, bproj_d, y_d, phases=phases)
    nc.compile()
    return nc


_NC = None


def kernel(x, w_qkv, w_proj, b_proj):
    global _NC
    if _NC is None:
        _NC = build_nc()
    x = np.ascontiguousarray(np.asarray(x, dtype=np.float32))
    w_qkv = np.ascontiguousarray(np.asarray(w_qkv, dtype=np.float32))
    w_proj = np.ascontiguousarray(np.asarray(w_proj, dtype=np.float32))
    b_proj = np.ascontiguousarray(np.asarray(b_proj, dtype=np.float32))
    in_maps = [
        {"x": x[c], "w_qkv": w_qkv, "w_proj": w_proj, "b_proj": b_proj}
        for c in range(NCORES)
    ]
    res = run_bass_kernel_spmd(_NC, in_maps, list(range(NCORES)))
    return np.stack([res.results[c]["y"] for c in range(NCORES)], axis=0)



# revision 6
# speedup vs baseline: 341.1686x; 341.1686x over previous
"""Trainium2 Bass kernel for nn_Attention (dense transformer MHA block).

Reference computation (per batch element b of 8):
    qkv = x @ w_qkv;  q,k,v split into 16 heads of dim 64
    out = softmax(q k^T / 8) v  (per head),  y = out @ w_proj + b_proj

Sharding: pure data-parallel over the batch (B=8 == n_cores). Each core
computes one batch element's full attention with replicated weights; no
collectives. Full inputs in, full outputs out; gather = np.stack.

Per-core dataflow (fp32 weights bitcast to f32r -- no convert copies):
  phase A: per n-tile, load x, PE-transpose into one big xT [d, n] tile
    (DVE grouped eviction); v-pass matmuls (xT-stationary, w_v-moving)
    packed into v_aug [n, 16*(64+1)] bf16 tiles with a ones column per
    head (integrated softmax denominator); v_aug evicted on Act.
  phase B: software-pipelined over heads. Per head-step h:
      emit S(h+1) per j-tile (kT-stationary, qT-moving, K=64) + exp on
      Act (PSUM->SBUF, bf16 out), woven with PV(h) accumulation steps
      (v_aug-stationary, P_T-moving) and, on even steps, the next
      pair's qk matmuls (w-stationary, xT-moving) + DVE evictions.
      Row 64 of PV = softmax denominator -> DVE reciprocal -> Pool
      partition_broadcast -> DVE multiply into attn_T [c, n] (f32r).
  proj: attn_T-stationary @ w_proj-moving; bias added during the
    PSUM->SBUF eviction via a pre-broadcast bias tile (DVE add).
"""

import numpy as np
from contextlib import ExitStack

import concourse.bass as bass
import concourse.bacc as bacc
import concourse.mybir as mybir
from concourse import tile
from concourse.bass_utils import run_bass_kernel_spmd
from concourse.masks import make_identity

F32 = mybir.dt.float32
F32R = mybir.dt.float32r
BF16 = mybir.dt.bfloat16
EXPF = mybir.ActivationFunctionType.Exp

MD = F32R        # matmul operand view dtype for fp32 data
VD = BF16        # v_aug / P_T dtype (PV matmul operands)

CONFIG = {
    "s_bufs": 2,      # S psum tiles in flight ([P, 1024] = 2 banks each)
    "mm_bufs": 2,     # shared matmul-out PSUM pool (1 bank each)
    "pv_bufs": 2,     # PV accumulator PSUM tiles (1 bank each)
    "p_bufs": 16,     # P_T sbuf tiles (2 heads in flight x 8 j-tiles)
}

N = 1024          # sequence length (per core)
D = 1024          # model dim
H = 16            # heads
HD = 64           # head dim
SCALE = HD ** -0.5
P = 128           # partitions
NT = N // P       # 8 n-tiles
DT = D // P       # 8 d-chunks
NCORES = 8


def _build(tc, nc, x_d, wqkv_d, wproj_d, bproj_d, y_d, phases="full"):
    mul = mybir.AluOpType.mult
    add = mybir.AluOpType.add

    with ExitStack() as outer:
        const = outer.enter_context(tc.tile_pool(name="const", bufs=1))
        bias_bc = const.tile([P, D], F32)

        attn_pool = outer.enter_context(tc.tile_pool(name="attnout", bufs=DT))
        attn_t = [attn_pool.tile([P, N], MD, tag="attn", name=f"attn{i}")
                  for i in range(DT)]
        outsb = outer.enter_context(tc.tile_pool(name="outsb", bufs=2))
        mm_ps = outer.enter_context(
            tc.tile_pool(name="mmps", bufs=CONFIG["mm_bufs"], space="PSUM"))

        xt_pool = outer.enter_context(tc.tile_pool(name="xT", bufs=1))
        xTall = xt_pool.tile([P, DT * N], MD, tag="xT", name="xTall")

        def xT(dt, lo, sz):
            return xTall[:, dt * N + lo: dt * N + lo + sz]

        vaug_pool = outer.enter_context(tc.tile_pool(name="vaug", bufs=NT))
        vaug = [vaug_pool.tile([P, H * (HD + 1)], VD, tag="vaug",
                               name=f"vaug{i}") for i in range(NT)]

        # ---- phase A: x load + transpose; v-pass into v_aug ----
        with ExitStack() as phA:
            scratch = phA.enter_context(tc.tile_pool(name="scratch", bufs=1))
            xload = phA.enter_context(tc.tile_pool(name="xload", bufs=3))
            wv_pool = phA.enter_context(tc.tile_pool(name="wvf", bufs=2 * DT))
            tp_ps = phA.enter_context(
                tc.tile_pool(name="tpps", bufs=2, space="PSUM"))

            ident_f = scratch.tile([P, P], F32)
            make_identity(nc, ident_f[:])
            ident = ident_f[:].bitcast(MD)

            # x tile 0 DMA ahead of the weight DMAs
            xf0 = xload.tile([P, D], F32)
            nc.sync.dma_start(xf0[:], x_d[0:P, :])

            wv = {}
            for cv in range(2):
                for dt in range(DT):
                    wf = wv_pool.tile([P, 512], F32, tag="wv", name=f"wv{cv}_{dt}")
                    nc.sync.dma_start(
                        wf[:], wqkv_d[dt * P:(dt + 1) * P,
                                      2 * D + cv * 512: 2 * D + (cv + 1) * 512])
                    wv[(cv, dt)] = wf

            bstage = scratch.tile([1, D], F32)
            nc.sync.dma_start(bstage[:], bproj_d[:].rearrange("(a f) -> a f", a=1))
            nc.gpsimd.partition_broadcast(bias_bc[:], bstage[:])

            for nt in range(NT):
                if nt == 0:
                    xfn = xf0
                else:
                    xfn = xload.tile([P, D], F32)
                    nc.sync.dma_start(xfn[:], x_d[nt * P:(nt + 1) * P, :])
                xr = xfn[:].bitcast(MD)
                for half in range(2):
                    tp = tp_ps.tile([P, 512], MD, tag="tp")
                    for q in range(4):
                        dt = half * 4 + q
                        nc.tensor.transpose(tp[:, q * P:(q + 1) * P],
                                            xr[:, dt * P:(dt + 1) * P], ident)
                    dst = xTall[:].rearrange("p (d n) -> p d n", d=DT)[
                        :, half * 4:(half + 1) * 4, nt * P:(nt + 1) * P]
                    src = tp[:].rearrange("p (d n) -> p d n", d=4)
                    nc.vector.tensor_copy(dst, src)
                nc.gpsimd.memset(
                    vaug[nt][:].rearrange("p (h e) -> p h e", h=H)[:, :, HD:HD + 1],
                    1.0)
                for cv in range(2):
                    vp = mm_ps.tile([P, 512], F32, tag="mm")
                    for dt in range(DT):
                        nc.tensor.matmul(
                            vp[:], xT(dt, nt * P, P), wv[(cv, dt)][:].bitcast(MD),
                            start=(dt == 0), stop=(dt == DT - 1))
                    dstv = vaug[nt][:].rearrange(
                        "p (h e) -> p h e", h=H)[:, 8 * cv:8 * cv + 8, 0:HD]
                    srcv = vp[:].rearrange("p (h e) -> p h e", h=8)
                    nc.scalar.copy(dstv, srcv)

        if phases == "A":
            for nt in range(NT):
                yo = outsb.tile([P, 512], F32, tag="y")
                nc.vector.tensor_copy(yo[:], vaug[nt][:, 0:512])
                nc.sync.dma_start(y_d[nt * P:(nt + 1) * P, 0:512], yo[:])
            return

        # ---- phase B: software-pipelined attention over heads ----
        with ExitStack() as phB:
            wqk_f = phB.enter_context(tc.tile_pool(name="wqkf", bufs=2 * DT))
            qk_pool = phB.enter_context(tc.tile_pool(name="qk", bufs=4))
            p_pool = phB.enter_context(
                tc.tile_pool(name="pT", bufs=CONFIG["p_bufs"]))
            s_ps = phB.enter_context(
                tc.tile_pool(name="sps", bufs=CONFIG["s_bufs"], space="PSUM"))
            pv_ps = phB.enter_context(
                tc.tile_pool(name="pvps", bufs=CONFIG["pv_bufs"], space="PSUM"))
            rt_pool = phB.enter_context(tc.tile_pool(name="rt", bufs=2))
            bt_pool = phB.enter_context(tc.tile_pool(name="bt", bufs=2))

            wqk = {}     # pair -> list of [P, 2, P] fp32 tiles
            qk_t = {}    # pair -> (qT, kT)
            p_t = {}     # (h, jt) -> pt tile

            def dma_wqk(pair):
                tiles = []
                for dt in range(DT):
                    wf = wqk_f.tile([P, 2, P], F32, tag="wqk", name=f"wqk{pair}_{dt}")
                    src = wqkv_d[dt * P:(dt + 1) * P, :].rearrange(
                        "p (k r e) -> p k r e", k=3, e=P)[:, 0:2, pair, :]
                    nc.sync.dma_start(wf[:], src)
                    tiles.append(wf)
                wqk[pair] = tiles

            qk_state = {}

            def emit_qk_piece(pair, step):
                # 8 steps; each emits 4 dt-matmuls of one accumulation
                # group (which, nch); groups change every 2 steps.
                which, nch = divmod(step // 2, 2)
                sub = step % 2
                if sub == 0:
                    if which == 0 and nch == 0:
                        qk_t[pair] = (
                            qk_pool.tile([P, N], MD, tag="qk", name=f"q{pair}"),
                            qk_pool.tile([P, N], MD, tag="qk", name=f"k{pair}"))
                    qk_state[pair] = mm_ps.tile([P, 512], F32, tag="mm",
                                                name=f"qp{pair}_{step}")
                qp = qk_state[pair]
                for i in range(4):
                    dt = sub * 4 + i
                    nc.tensor.matmul(
                        qp[:], wqk[pair][dt][:, which, :].bitcast(MD),
                        xT(dt, nch * 512, 512),
                        start=(dt == 0), stop=(dt == DT - 1))
                if sub == 1:
                    ct = qk_t[pair][which]
                    nc.vector.tensor_copy(ct[:, nch * 512:(nch + 1) * 512], qp[:])

            def emit_S(h, jt):
                pair, hh = divmod(h, 2)
                base = HD * hh
                qT, kT = qk_t[pair]
                sp = s_ps.tile([P, N], F32, tag="s")
                for ich in range(2):
                    nc.tensor.matmul(
                        sp[:, ich * 512:(ich + 1) * 512],
                        kT[base:base + HD, jt * P:(jt + 1) * P],
                        qT[base:base + HD, ich * 512:(ich + 1) * 512],
                        start=True, stop=True)
                pt = p_pool.tile([P, N], VD, tag="p", name=f"pT{h}_{jt}")
                nc.scalar.activation(pt[:], sp[:], EXPF, scale=SCALE)
                p_t[(h, jt)] = pt

            def emit_norm(h, pvs):
                pair, hh = divmod(h, 2)
                base = HD * hh
                rt = rt_pool.tile([1, N], F32, tag="rt")
                bt = bt_pool.tile([HD, N], F32, tag="bt")
                for ich in range(2):
                    nc.vector.reciprocal(rt[:, ich * 512:(ich + 1) * 512],
                                         pvs[ich][HD:HD + 1, :])
                nc.gpsimd.partition_broadcast(bt[:], rt[:])
                for ich in range(2):
                    nc.vector.tensor_tensor(
                        attn_t[pair][base:base + HD, ich * 512:(ich + 1) * 512],
                        pvs[ich][0:HD, :], bt[:, ich * 512:(ich + 1) * 512], mul)

            dma_wqk(0)
            dma_wqk(1)
            for step in range(NT):
                emit_qk_piece(0, step)
            for jt in range(NT):
                emit_S(0, jt)

            for h in range(H):
                pair = h // 2
                if h % 2 == 0 and pair + 2 < H // 2:
                    dma_wqk(pair + 2)
                pvs = [pv_ps.tile([HD + 1, 512], F32, tag="pv",
                                  name=f"pv{h}_{i}") for i in range(2)]
                for jt in range(NT):
                    if h + 1 < H:
                        emit_S(h + 1, jt)
                    if h % 2 == 0 and pair + 1 < H // 2:
                        emit_qk_piece(pair + 1, jt)
                    for ich in range(2):
                        nc.tensor.matmul(
                            pvs[ich][:],
                            vaug[jt][:, h * (HD + 1):(h + 1) * (HD + 1)],
                            p_t[(h, jt)][:, ich * 512:(ich + 1) * 512],
                            start=(jt == 0), stop=(jt == NT - 1))
                emit_norm(h, pvs)

        if phases == "AB":
            for cc in range(DT):
                yo = outsb.tile([P, 512], F32, tag="y")
                nc.vector.tensor_copy(yo[:], attn_t[cc][:, 0:512])
                nc.sync.dma_start(y_d[cc * P:(cc + 1) * P, 0:512], yo[:])
            return

        # ---- projection (+ bias via DVE add during eviction) ----
        with ExitStack() as ph3:
            wp_f = ph3.enter_context(tc.tile_pool(name="wpf", bufs=2 * DT))
            wp = {}
            for cc in range(DT):
                for ec in range(2):
                    wf = wp_f.tile([P, 512], F32, tag="wp", name=f"wp{cc}_{ec}")
                    nc.sync.dma_start(
                        wf[:], wproj_d[cc * P:(cc + 1) * P,
                                       ec * 512:(ec + 1) * 512])
                    wp[(cc, ec)] = wf
            for nt in range(NT):
                for ec in range(2):
                    yp = mm_ps.tile([P, 512], F32, tag="mm")
                    for cc in range(DT):
                        nc.tensor.matmul(
                            yp[:], attn_t[cc][:, nt * P:(nt + 1) * P],
                            wp[(cc, ec)][:].bitcast(MD), start=(cc == 0),
                            stop=(cc == DT - 1))
                    yo = outsb.tile([P, 512], F32, tag="y")
                    nc.vector.tensor_tensor(
                        yo[:], yp[:], bias_bc[:, ec * 512:(ec + 1) * 512], add)
                    nc.sync.dma_start(
                        y_d[nt * P:(nt + 1) * P, ec * 512:(ec + 1) * 512], yo[:])


def build_nc(reps=1, phases="full"):
    nc = bacc.Bacc("TRN2", target_bir_lowering=False, debug=False)
    x_d = nc.dram_tensor("x", [N, D], F32, kind="ExternalInput").ap()
    wqkv_d = nc.dram_tensor("w_qkv", [D, 3 * D], F32, kind="ExternalInput").ap()
    wproj_d = nc.dram_tensor("w_proj", [D, D], F32, kind="ExternalInput").ap()
    bproj_d = nc.dram_tensor("b_proj", [D], F32, kind="ExternalInput").ap()
    y_d = nc.dram_tensor("y", [N, D], F32, kind="ExternalOutput").ap()
    with tile.TileContext(nc) as tc:
        for _ in range(reps):
            _build(tc, nc, x_d, wqkv_d, wproj_d, bproj_d, y_d, phases=phases)
    nc.compile()
    return nc


_NC = None


def kernel(x, w_qkv, w_proj, b_proj):
    global _NC
    if _NC is None:
        _NC = build_nc()
    x = np.ascontiguousarray(np.asarray(x, dtype=np.float32))
    w_qkv = np.ascontiguousarray(np.asarray(w_qkv, dtype=np.float32))
    w_proj = np.ascontiguousarray(np.asarray(w_proj, dtype=np.float32))
    b_proj = np.ascontiguousarray(np.asarray(b_proj, dtype=np.float32))
    in_maps = [
        {"x": x[c], "w_qkv": w_qkv, "w_proj": w_proj, "b_proj": b_proj}
        for c in range(NCORES)
    ]
    res = run_bass_kernel_spmd(_NC, in_maps, list(range(NCORES)))
    return np.stack([res.results[c]["y"] for c in range(NCORES)], axis=0)
